# revision 4
# baseline (speedup 1.0000x reference)
"""CharRNN (Elman) recurrence kernel for 8 trn2 NeuronCores.

Strategy: the 8192-step recurrence h = tanh(xp[t] + Waa @ h) is the only
serial part; everything else (embedding gather, input projection, output
projection) is computed on the host in numpy.  The device kernel runs the
recurrence tensor-parallel over the hidden dim: each core owns 256 output
rows (2 chunks of 128 partitions), computes them with 32 stationary-weight
matmuls per step (PSUM accumulated over 16 K-chunks of 128), applies
tanh+bias on ScalarE, and broadcasts its [128,2] shard to the other 7 cores
via SWDGE remote_dma_broadcast (XOR-relative destinations).  Per-peer
semaphores with monotonic thresholds synchronize arrivals; h is
double-buffered by step parity.

Cross-core addressing (measured on hw): a broadcast with relative dest
(0, d) from core s lands on core s ^ d ^ (2 if d & 4 else 0) -- the ^2 is
the cross-die D2D twist.  Each core's weight data is permuted on the host
so that h-buffer slot d always holds the pair from that sender.
"""

import numpy as np

SEQ = 8192
HID = 2048
NCORES = 8
BLK = 128
EFF = [d ^ 2 if d & 4 else d for d in range(8)]  # relative-dest wire twist


def _mybir():
    import concourse.mybir as mybir

    return mybir


def _dt(w_dt):
    mybir = _mybir()
    return mybir.dt.float32 if w_dt == "f32" else mybir.dt.bfloat16


def _np_dt(w_dt):
    if w_dt == "f32":
        return np.float32
    import ml_dtypes

    return ml_dtypes.bfloat16


def build(seq=SEQ, w_dt="f32", repeat=1):
    """Build + compile the SPMD recurrence program. Returns nc."""
    import concourse.bacc as bacc

    mybir = _mybir()
    wdt = _dt(w_dt)
    nblk = seq // BLK
    T = repeat * seq
    GBLK = T // BLK

    nc = bacc.Bacc("TRN2", target_bir_lowering=False, debug=False,
                   num_devices=NCORES)

    wt_in = nc.dram_tensor("wt", [128, 32 * 128], wdt, kind="ExternalInput").ap()
    xp_in = nc.dram_tensor("xp", [nblk, 128, 2 * BLK], mybir.dt.float32,
                           kind="ExternalInput").ap()
    hs_out = nc.dram_tensor("hs", [nblk, 128, 2 * BLK], wdt,
                            kind="ExternalOutput").ap()

    wt_sb = nc.alloc_sbuf_tensor("wt_sb", [128, 32 * 128], wdt).ap()
    xp_sb = [nc.alloc_sbuf_tensor(f"xp_sb{i}", [128, 2 * BLK],
                                  mybir.dt.float32).ap() for i in range(2)]
    hs_sb = [nc.alloc_sbuf_tensor(f"hs_sb{i}", [128, 2 * BLK], wdt).ap()
             for i in range(2)]
    h_buf = [nc.alloc_sbuf_tensor(f"h_buf{i}", [128, 16], wdt).ap()
             for i in range(2)]
    psum = [[nc.alloc_psum_tensor(f"ps{p}{mc}", [128, 1],
                                  mybir.dt.float32).ap()
             for mc in range(2)] for p in range(2)]

    wt_sem = nc.alloc_semaphore("wt_sem")
    init_sem = nc.alloc_semaphore("init_sem")
    xp_sem = nc.alloc_semaphore("xp_sem")
    st_sem = nc.alloc_semaphore("st_sem")
    act_sem = nc.alloc_semaphore("act_sem")
    dve_sem = nc.alloc_semaphore("dve_sem")
    mm_sem = [nc.alloc_semaphore(f"mm_sem{i}") for i in range(2)]
    # 16-bit sem counters: rotate high-rate sems so none exceeds 65535.
    prep_sems = [nc.alloc_semaphore(f"prep_sem{i}") for i in range(4)]
    snd_sems = [nc.alloc_semaphore(f"snd_sem{i}") for i in range(16)]
    recv = [None] + [nc.alloc_semaphore(f"recv{d}") for d in range(1, 8)]

    g = nc.gpsimd
    sp = nc.sync
    pe = nc.tensor
    se = nc.scalar
    ve = nc.vector
    Tanh = mybir.ActivationFunctionType.Tanh

    # ---- init ----
    g.memset(h_buf[0][:], 0.0).then_inc(init_sem, 1)
    g.memset(h_buf[1][:], 0.0).then_inc(init_sem, 1)
    sp.dma_start(wt_sb[:], wt_in[:]).then_inc(wt_sem, 16)
    sp.dma_start(xp_sb[0][:], xp_in[0]).then_inc(xp_sem, 16)
    if GBLK > 1:
        sp.dma_start(xp_sb[1][:], xp_in[1 % nblk]).then_inc(xp_sem, 16)
    nc.all_core_barrier()
    pe.wait_ge(wt_sem, 16)
    pe.wait_ge(init_sem, 2)

    for t in range(T):
        td = t % seq
        gB = t // BLK
        sb = td % BLK
        P = t % 2
        NP = 1 - P
        hsb = hs_sb[gB % 2]
        xsb = xp_sb[gB % 2]

        # ---- SP: block boundary ----
        if sb == 0 and t > 0:
            sp.wait_ge(act_sem, 2 * BLK * gB)
            prevB = gB - 1
            sp.dma_start(hs_out[prevB % nblk],
                         hs_sb[prevB % 2][:]).then_inc(st_sem, 16)
            if gB + 1 < GBLK:
                sp.dma_start(xp_sb[(gB + 1) % 2][:],
                             xp_in[(gB + 1) % nblk]).then_inc(xp_sem, 16)

        # ---- PE: 32 matmuls ----
        if t >= 2:
            pe.wait_ge(act_sem, 2 * (t - 1))  # psum parity free
        for mc in range(2):
            pos = 0
            for d in range(8):
                for m in range(2):
                    tile = mc * 16 + d * 2 + m
                    ins = pe.matmul(
                        psum[P][mc],
                        wt_sb[:, tile * 128:(tile + 1) * 128],
                        h_buf[P][:, d * 2 + m:d * 2 + m + 1],
                        start=(pos == 0), stop=(pos == 15),
                    )
                    if t > 0 and mc == 0 and m == 0:
                        if d == 0:
                            ins.wait_op(dve_sem, t, "sem-ge")
                        else:
                            ins.wait_op(recv[d], 2 * t, "sem-ge")
                    if pos == 15:
                        ins.then_inc(mm_sem[mc], 1)
                    pos += 1

        # ---- ACT: tanh + bias ----
        if sb == 0:
            se.wait_ge(xp_sem, 16 * (gB + 1))
            if gB >= 2:
                se.wait_ge(st_sem, 16 * (gB - 1))
        for mc in range(2):
            a = se.activation(hsb[:, sb * 2 + mc:sb * 2 + mc + 1],
                              psum[P][mc], Tanh,
                              bias=xsb[:, sb * 2 + mc:sb * 2 + mc + 1])
            a.wait_op(mm_sem[mc], t + 1, "sem-ge")
            a.then_inc(act_sem, 1)

        # ---- DVE: own shard -> next-parity h_buf slot 0 ----
        c = ve.tensor_copy(h_buf[NP][:, 0:2], hsb[:, sb * 2:sb * 2 + 2])
        c.wait_op(act_sem, 2 * (t + 1), "sem-ge")
        c.then_inc(dve_sem, 1)

        # ---- GPSIMD: descgen for step t sends; fire step t-1 sends ----
        for d in range(1, 8):
            rd = [None] * 8
            rd[d] = (0, d)
            g.remote_dma_broadcast(
                h_buf[NP][:, d * 2:d * 2 + 2],
                hsb[:, sb * 2:sb * 2 + 2],
                remote_sem=recv[d], local_sem=snd_sems[t % 16], rdests=rd,
            ).then_inc(prep_sems[t % 4], 1)
        if t > 0:
            pt = t - 1
            g.wait_ge(prep_sems[pt % 4], 7 * (pt // 4 + 1))
            g.wait_ge(act_sem, 2 * t)
            g.trigger_dma(count=7)

    # ---- final ----
    pt = T - 1
    g.wait_ge(prep_sems[pt % 4], 7 * (pt // 4 + 1))
    g.wait_ge(act_sem, 2 * T)
    g.trigger_dma(count=7)
    for d in range(1, 8):
        g.wait_ge(recv[d], 2 * T)
    sp.wait_ge(act_sem, 2 * T)
    lastB = GBLK - 1
    sp.dma_start(hs_out[lastB % nblk], hs_sb[lastB % 2][:]).then_inc(st_sem, 16)
    sp.wait_ge(st_sem, 16 * GBLK)

    nc.compile()
    return nc


def prep_core_inputs(Waa, xp, w_dt="f32", seq=SEQ):
    """Per-core input dicts. xp: [seq, HID] f32."""
    npdt = _np_dt(w_dt)
    nblk = seq // BLK
    WaaT_c = np.ascontiguousarray(Waa.astype(npdt))
    xp4 = xp.astype(np.float32).reshape(nblk, BLK, 16, 128)
    ins = []
    for k in range(NCORES):
        wt = np.empty((128, 32, 128), npdt)
        for mc in range(2):
            i0 = (2 * k + mc) * 128
            for d in range(8):
                s = k ^ EFF[d]
                for m in range(2):
                    c0 = (2 * s + m) * 128
                    tile = mc * 16 + d * 2 + m
                    wt[:, tile, :] = WaaT_c[i0:i0 + 128, c0:c0 + 128].T
        sl = xp4[:, :, 2 * k:2 * k + 2, :]          # [b, s, mc, p]
        xpk = np.ascontiguousarray(
            np.transpose(sl, (0, 3, 1, 2)).reshape(nblk, 128, 2 * BLK))
        ins.append({"wt": wt.reshape(128, 32 * 128), "xp": xpk})
    return ins


def assemble_hs(results, w_dt="f32", seq=SEQ):
    """results: list of per-core out dicts -> hs [seq, HID] f32."""
    nblk = seq // BLK
    hs4 = np.empty((nblk, BLK, 16, 128), np.float32)
    for k in range(NCORES):
        r = results[k]["hs"].astype(np.float32).reshape(nblk, 128, BLK, 2)
        hs4[:, :, 2 * k:2 * k + 2, :] = np.transpose(r, (0, 2, 3, 1))
    return hs4.reshape(seq, HID)


_CACHE = {}


def _get_nc(seq, w_dt, repeat):
    key = (seq, w_dt, repeat)
    if key not in _CACHE:
        _CACHE[key] = build(seq, w_dt, repeat)
    return _CACHE[key]


def run_device(Waa, xp, w_dt="f32", seq=SEQ, repeat=1):
    from concourse import bass_utils

    nc = _get_nc(seq, w_dt, repeat)
    ins = prep_core_inputs(Waa, xp, w_dt, seq)
    import time

    t0 = time.time()
    res = bass_utils.run_bass_kernel_spmd(nc, ins, core_ids=list(range(NCORES)))
    wall = time.time() - t0
    return assemble_hs(res.results, w_dt, seq), wall


W_DT = "f32"


def kernel(input_seq, emb_table, Wax, bax, Waa, Wya, bya):
    input_seq = np.asarray(input_seq).astype(np.int64)
    emb_table = np.asarray(emb_table, np.float32)
    Wax = np.asarray(Wax, np.float32)
    bax = np.asarray(bax, np.float32)
    Waa = np.asarray(Waa, np.float32)
    Wya = np.asarray(Wya, np.float32)
    bya = np.asarray(bya, np.float32)

    embs = emb_table[input_seq]                      # [S, E]
    xp = embs @ Wax.T + bax                          # [S, H]
    hs, _ = run_device(Waa, xp, W_DT, SEQ, 1)        # [S, H]
    ys = hs @ Wya.T + bya                            # [S, V]
    h_last = hs[-1]
    return ys, h_last
